# revision 5
# baseline (speedup 1.0000x reference)
"""Multi-head attention (B=1, S=4096, D=768, H=12) on 8 Trainium2 NeuronCores.

Sharding: queries are split 8 x 512 across cores (data parallel over the
query/sequence axis). Each core computes all 12 heads for its 512 queries:
full K/V projections are recomputed per core (cheap vs. the 100MB/core attn
writeback, which dominates: target_regime=memory).

Per-core pipeline (all matmuls bf16 into f32 PSUM):
  phase 0: W/bias load, bf16 convert, DMA-xbar transpose -> WT [d, hdh]
  phase 1: Q chunk -> QT -> pQT [hdh, q] (+bq via K=1 ones-matmul)
  phase 2: stream K/V in 512-row chunks -> KT/VT -> pKT [hdh, k], pV [k, hdh]
           (pV columns interleaved per head with a ones column: [V_h | 1])
  phase 3: per head h:
    A: scoresT tiles [k,q] = pK_h^T pQ_h -> exp (ACT, scale=1/64) -> UT bf16
       -> OT[65,512] += [pV_h | 1]^T UT   (row 64 = softmax denominators)
       -> OTall[h] = OT[0:64] * (1/denom) broadcast  (normalized attn @ V)
    B: scores tiles [q,k] -> exp with accum_out (row sums) -> U [q,4096] f32
       -> U *= 1/rowsum (DVE, per-partition scalar) -> DMA to attn output
  phase 4: out = OTall^T @ Wo^T + bo -> DMA

kernel(**inputs) takes the full problem inputs and returns (out, attn) like
the reference: out [1,4096,768] f32, attn [1,12,4096,4096] f32.
"""

import numpy as np

import concourse.bacc as bacc
import concourse.bass as bass
import concourse.mybir as mybir
import concourse.tile as tile
from concourse.bass_utils import run_bass_kernel_spmd

F32 = mybir.dt.float32
BF16 = mybir.dt.bfloat16
EXP = mybir.ActivationFunctionType.Exp

B, S, D, H = 1, 4096, 768, 12
DH = D // H  # 64
CORES = 8
SQ = S // CORES  # 512 queries per core
DT = D // 128  # 6 d-tiles
QT = SQ // 128  # 4 q-tiles
KTN = S // 128  # 32 k-tiles
KCN = S // 512  # 8 k-chunks of 512
PVW = H * (DH + 1)  # 780: per-head 64 V columns + 1 ones column

# psum scores group: SC_G k-tiles/chunks of 512 each ([128, SC_G*512] f32)
SC_G = 3


def _groups(n, g):
    out = []
    i = 0
    while i < n:
        out.append((i, min(g, n - i)))
        i += g
    return out


def build_nc():
    nc = bacc.Bacc("TRN2", target_bir_lowering=False, debug=False,
                   num_devices=CORES)

    Qc = nc.dram_tensor("Qc", [SQ, D], F32, kind="ExternalInput").ap()
    Kt = nc.dram_tensor("K", [S, D], F32, kind="ExternalInput").ap()
    Vt = nc.dram_tensor("V", [S, D], F32, kind="ExternalInput").ap()
    Ws = {w: nc.dram_tensor(w, [D, D], F32, kind="ExternalInput").ap()
          for w in ("Wq", "Wk", "Wv", "Wo")}
    bs = {b_: nc.dram_tensor(b_, [D], F32, kind="ExternalInput").ap()
          for b_ in ("bq", "bk", "bv", "bo")}
    attn_c = nc.dram_tensor("attn_c", [H, SQ, S], F32, kind="ExternalOutput").ap()
    out_c = nc.dram_tensor("out_c", [SQ, D], F32, kind="ExternalOutput").ap()

    with tile.TileContext(nc) as tc:
        with tc.tile_pool(name="persist", bufs=1) as persist:
            pKTb = persist.tile([128, DT, S], BF16)      # [d-in, hdh-out, k]
            pVb = persist.tile([128, KTN, PVW], BF16)    # [k-in, k-out, h*(65)]
            pQTb = persist.tile([128, DT, SQ], BF16)
            WoTb = persist.tile([128, DT, D], BF16)      # [hdh-in, hdh-out, dout]
            OTall = persist.tile([128, DT, SQ], BF16)    # [hdh-in, hdh-out, q]
            ones_bf = persist.tile([1, 512], BF16)
            brow = {b_: persist.tile([1, D], BF16, name=f"brow_{b_}")
                    for b_ in ("bq", "bk", "bv", "bo")}

            nc.vector.memset(ones_bf, 1.0)
            # ones columns interleaved in pV: pVb[:, :, h*65 + 64] = 1
            nc.vector.memset(
                pVb.rearrange("p k (h c) -> p k h c", c=DH + 1)[:, :, :, DH], 1.0)

            # ---- phase 0-2: projections ----
            with tc.tile_pool(name="wtmp", bufs=3) as wtmp, \
                 tc.tile_pool(name="wT", bufs=1) as wT, \
                 tc.tile_pool(name="kvc", bufs=2) as kvc, \
                 tc.tile_pool(name="pp", bufs=4, space="PSUM") as pp:

                # bias rows -> bf16 [1, D]
                for b_ in ("bq", "bk", "bv", "bo"):
                    bf = wtmp.tile([1, D], F32, tag="brow_f32")
                    nc.sync.dma_start(out=bf, in_=bs[b_].rearrange("(a d) -> a d", a=1))
                    nc.vector.tensor_copy(brow[b_], bf)

                # W transposes: WT[p, t, c] = W[c, 128 t + p]
                WTs = {}
                for w in ("Wq", "Wk", "Wv"):
                    WTs[w] = wT.tile([128, DT, D], BF16, tag=f"{w}T", name=f"{w}T")
                WTs["Wo"] = WoTb
                for w in ("Wq", "Wk", "Wv", "Wo"):
                    for r in range(DT):
                        wf = wtmp.tile([128, D], F32, tag="w_f32")
                        nc.sync.dma_start(out=wf, in_=Ws[w][r * 128:(r + 1) * 128, :])
                        wb = wtmp.tile([128, D], BF16, tag="w_bf16")
                        nc.vector.tensor_copy(wb, wf)
                        nc.sync.dma_start_transpose(
                            out=WTs[w][:, :, r * 128:(r + 1) * 128], in_=wb)

                # Q -> QTb [128, DT, SQ]
                QTb = wT.tile([128, DT, SQ], BF16, tag="QTb")
                for i in range(QT):
                    qf = wtmp.tile([128, D], F32, tag="w_f32")
                    nc.sync.dma_start(out=qf, in_=Qc[i * 128:(i + 1) * 128, :])
                    qb = wtmp.tile([128, D], BF16, tag="w_bf16")
                    nc.vector.tensor_copy(qb, qf)
                    nc.sync.dma_start_transpose(
                        out=QTb[:, :, i * 128:(i + 1) * 128], in_=qb)

                # pQT = Wq^T-contraction + bq
                for hb in range(DT):
                    ps = pp.tile([128, SQ], F32, tag="proj")
                    nc.tensor.matmul(ps, brow["bq"][:, hb * 128:(hb + 1) * 128],
                                     ones_bf[:, :SQ], start=True, stop=False)
                    for t in range(DT):
                        nc.tensor.matmul(ps, WTs["Wq"][:, t, hb * 128:(hb + 1) * 128],
                                         QTb[:, t, :], start=False, stop=(t == DT - 1))
                    nc.vector.tensor_copy(pQTb[:, hb, :], ps)

                # K/V streamed per 512-chunk
                for kc in range(KCN):
                    KTc = kvc.tile([128, DT, 512], BF16, tag="KTc")
                    VTc = kvc.tile([128, DT, 512], BF16, tag="VTc")
                    for i in range(4):
                        r0 = kc * 512 + i * 128
                        for (src, dst) in ((Kt, KTc), (Vt, VTc)):
                            xf = wtmp.tile([128, D], F32, tag="w_f32")
                            nc.sync.dma_start(out=xf, in_=src[r0:r0 + 128, :])
                            xb = wtmp.tile([128, D], BF16, tag="w_bf16")
                            nc.vector.tensor_copy(xb, xf)
                            nc.sync.dma_start_transpose(
                                out=dst[:, :, i * 128:(i + 1) * 128], in_=xb)
                    # pKT chunk
                    for hb in range(DT):
                        ps = pp.tile([128, 512], F32, tag="proj")
                        nc.tensor.matmul(ps, brow["bk"][:, hb * 128:(hb + 1) * 128],
                                         ones_bf[:, :512], start=True, stop=False)
                        for t in range(DT):
                            nc.tensor.matmul(ps, WTs["Wk"][:, t, hb * 128:(hb + 1) * 128],
                                             KTc[:, t, :], start=False, stop=(t == DT - 1))
                        nc.vector.tensor_copy(
                            pKTb[:, hb, kc * 512:(kc + 1) * 512], ps)
                    # pV chunk (4 k-subtiles x 2 hdh-halves)
                    pVb4 = pVb.rearrange("p k (h c) -> p k h c", c=DH + 1)
                    for ki in range(4):
                        for nb in range(2):
                            ps = pp.tile([128, 384], F32, tag="pvproj")
                            nc.tensor.matmul(ps, ones_bf[:, :128],
                                             brow["bv"][:, nb * 384:(nb + 1) * 384],
                                             start=True, stop=False)
                            for t in range(DT):
                                nc.tensor.matmul(
                                    ps, VTc[:, t, ki * 128:(ki + 1) * 128],
                                    WTs["Wv"][:, t, nb * 384:(nb + 1) * 384],
                                    start=False, stop=(t == DT - 1))
                            nc.vector.tensor_copy(
                                pVb4[:, kc * 4 + ki, nb * 6:(nb + 1) * 6, 0:DH], ps)

            # ---- phase 3: attention per head ----
            with tc.tile_pool(name="sc", bufs=2, space="PSUM") as scp, \
                 tc.tile_pool(name="ot", bufs=2, space="PSUM") as otp, \
                 tc.tile_pool(name="ut", bufs=3) as utp, \
                 tc.tile_pool(name="u", bufs=2) as up, \
                 tc.tile_pool(name="small", bufs=4) as small, \
                 tc.tile_pool(name="dsc", bufs=2, space="DRAM") as dscp:
                for h in range(H):
                    po = (h % 2) * 64
                    hb = h // 2
                    pq = pQTb[po:po + 64, hb, :]  # [64, SQ]

                    # --- A: UT tiles + attn@V with denominators ---
                    ot_ps = otp.tile([DH + 1, SQ], F32, tag="ot")
                    for (k0, gl) in _groups(KTN, SC_G):
                        sc_ps = scp.tile([128, SC_G * 512], F32, tag="sc")
                        ut_sb = utp.tile([128, SC_G * 512], BF16, tag="ut")
                        for j in range(gl):
                            kt = k0 + j
                            nc.tensor.matmul(
                                sc_ps[:, j * 512:(j + 1) * 512],
                                pKTb[po:po + 64, hb, kt * 128:(kt + 1) * 128],
                                pq, start=True, stop=True)
                        nc.scalar.activation(ut_sb[:, :gl * 512], sc_ps[:, :gl * 512],
                                             EXP, scale=float(1.0 / DH))
                        for j in range(gl):
                            kt = k0 + j
                            nc.tensor.matmul(
                                ot_ps, pVb[:, kt, h * 65:(h + 1) * 65],
                                ut_sb[:, j * 512:(j + 1) * 512],
                                start=(kt == 0), stop=(kt == KTN - 1))
                    rrow = small.tile([1, SQ], F32, tag="rrow")
                    nc.vector.reciprocal(rrow, ot_ps[DH:DH + 1, :])
                    rdram = dscp.tile([1, SQ], F32, tag="rdram")
                    nc.sync.dma_start(out=rdram, in_=rrow)
                    bcast = small.tile([64, SQ], F32, tag="bcast")
                    nc.sync.dma_start(out=bcast, in_=rdram.partition_broadcast(64))
                    nc.vector.tensor_mul(OTall[po:po + 64, hb, :],
                                         ot_ps[0:DH, :], bcast)

                    # --- B: attn rows + DMA out ---
                    for qt in range(QT):
                        u_sb = up.tile([128, S], F32, tag="u")
                        parts = small.tile([128, 4], F32, tag="parts")
                        gs = _groups(KCN, SC_G)
                        for gi, (c0, gl) in enumerate(gs):
                            sc_ps = scp.tile([128, SC_G * 512], F32, tag="sc")
                            for j in range(gl):
                                c = c0 + j
                                nc.tensor.matmul(
                                    sc_ps[:, j * 512:(j + 1) * 512],
                                    pQTb[po:po + 64, hb, qt * 128:(qt + 1) * 128],
                                    pKTb[po:po + 64, hb, c * 512:(c + 1) * 512],
                                    start=True, stop=True)
                            nc.scalar.activation(
                                u_sb[:, c0 * 512:(c0 + gl) * 512],
                                sc_ps[:, :gl * 512], EXP, scale=float(1.0 / DH),
                                accum_out=parts[:, gi:gi + 1])
                        sums = small.tile([128, 1], F32, tag="sums")
                        nc.vector.reduce_sum(sums, parts[:, 0:len(gs)],
                                             axis=mybir.AxisListType.X)
                        recip = small.tile([128, 1], F32, tag="recip")
                        nc.vector.reciprocal(recip, sums)
                        nc.vector.tensor_scalar_mul(u_sb, u_sb, recip)
                        nc.sync.dma_start(
                            out=attn_c[h, qt * 128:(qt + 1) * 128, :], in_=u_sb)

            # ---- phase 4: output projection ----
            with tc.tile_pool(name="p4", bufs=4, space="PSUM") as p4, \
                 tc.tile_pool(name="o4", bufs=2) as o4:
                for qt in range(QT):
                    osb = o4.tile([128, D], F32, tag="osb")
                    for nb in range(2):
                        ps = p4.tile([128, 384], F32, tag="p4")
                        nc.tensor.matmul(ps, ones_bf[:, :128],
                                         brow["bo"][:, nb * 384:(nb + 1) * 384],
                                         start=True, stop=False)
                        for t in range(DT):
                            nc.tensor.matmul(
                                ps, OTall[:, t, qt * 128:(qt + 1) * 128],
                                WoTb[:, t, nb * 384:(nb + 1) * 384],
                                start=False, stop=(t == DT - 1))
                        nc.vector.tensor_copy(osb[:, nb * 384:(nb + 1) * 384], ps)
                    nc.sync.dma_start(out=out_c[qt * 128:(qt + 1) * 128, :], in_=osb)

    nc.compile()
    return nc


_NC = None


def _get_nc():
    global _NC
    if _NC is None:
        _NC = build_nc()
    return _NC


def kernel(Q, K, V, Wq, bq, Wk, bk, Wv, bv, Wo, bo):
    nc = _get_nc()
    Q = np.ascontiguousarray(np.asarray(Q, dtype=np.float32))
    shared = {
        "K": np.ascontiguousarray(np.asarray(K, np.float32)[0]),
        "V": np.ascontiguousarray(np.asarray(V, np.float32)[0]),
        "Wq": np.ascontiguousarray(np.asarray(Wq, np.float32)),
        "Wk": np.ascontiguousarray(np.asarray(Wk, np.float32)),
        "Wv": np.ascontiguousarray(np.asarray(Wv, np.float32)),
        "Wo": np.ascontiguousarray(np.asarray(Wo, np.float32)),
        "bq": np.ascontiguousarray(np.asarray(bq, np.float32)),
        "bk": np.ascontiguousarray(np.asarray(bk, np.float32)),
        "bv": np.ascontiguousarray(np.asarray(bv, np.float32)),
        "bo": np.ascontiguousarray(np.asarray(bo, np.float32)),
    }
    in_maps = []
    for c in range(CORES):
        m = dict(shared)
        m["Qc"] = np.ascontiguousarray(Q[0, c * SQ:(c + 1) * SQ, :])
        in_maps.append(m)
    res = run_bass_kernel_spmd(nc, in_maps, core_ids=list(range(CORES)))
    out = np.empty((1, S, D), np.float32)
    attn = np.empty((1, H, S, S), np.float32)
    for c in range(CORES):
        out[0, c * SQ:(c + 1) * SQ, :] = res.results[c]["out_c"]
        attn[0, :, c * SQ:(c + 1) * SQ, :] = res.results[c]["attn_c"]
    return (out, attn)


if __name__ == "__main__":
    nc = _get_nc()
    print("built+compiled ok")


# revision 12
# speedup vs baseline: 351.6511x; 351.6511x over previous
"""Multi-head attention (B=1, S=4096, D=768, H=12) on 8 Trainium2 NeuronCores.

Sharding: queries are split 8 x 512 across cores (data parallel over the
query/sequence axis). Each core computes all 12 heads for its 512 queries:
full K/V projections are recomputed per core (cheap vs. the 100MB/core attn
writeback, which dominates: target_regime=memory).

Per-core pipeline (all matmuls bf16 into f32 PSUM):
  phase 0: W/bias load, bf16 convert, DMA-xbar transpose -> WT [d, hdh]
  phase 1: Q chunk -> QT -> pQT [hdh, q] (+bq via K=1 ones-matmul)
  phase 2: stream K/V in 512-row chunks -> KT/VT -> pKT [hdh, k], pV [k, hdh]
           (pV columns interleaved per head with a ones column: [V_h | 1])
  phase 3: per head h:
    A: scoresT tiles [k,q] = pK_h^T pQ_h -> exp (ACT, scale=1/64) -> UT bf16
       -> OT[65,512] += [pV_h | 1]^T UT   (row 64 = softmax denominators)
       -> OTall[h] = OT[0:64] * (1/denom) broadcast  (normalized attn @ V)
    B: scores tiles [q,k] -> exp with accum_out (row sums) -> U [q,4096] f32
       -> U *= 1/rowsum (DVE, per-partition scalar) -> DMA to attn output
  phase 4: out = OTall^T @ Wo^T + bo -> DMA

kernel(**inputs) takes the full problem inputs and returns (out, attn) like
the reference: out [1,4096,768] f32, attn [1,12,4096,4096] f32.
"""

import numpy as np

import concourse.bacc as bacc
import concourse.bass as bass
import concourse.mybir as mybir
import concourse.tile as tile

F32 = mybir.dt.float32
BF16 = mybir.dt.bfloat16
EXP = mybir.ActivationFunctionType.Exp

B, S, D, H = 1, 4096, 768, 12
DH = D // H  # 64
CORES = 8
SQ = S // CORES  # 512 queries per core
DT = D // 128  # 6 d-tiles
QT = SQ // 128  # 4 q-tiles
KTN = S // 128  # 32 k-tiles
KCN = S // 512  # 8 k-chunks of 512
PVW = H * (DH + 1)  # 780: per-head 64 V columns + 1 ones column

# psum scores group: SC_G k-tiles/chunks of 512 each ([128, SC_G*512] f32)
SC_G = 3


def _groups(n, g):
    out = []
    i = 0
    while i < n:
        out.append((i, min(g, n - i)))
        i += g
    return out


def build_nc():
    nc = bacc.Bacc("TRN2", target_bir_lowering=False, debug=False,
                   num_devices=CORES)

    Qc = nc.dram_tensor("Qc", [SQ, D], F32, kind="ExternalInput").ap()
    Kt = nc.dram_tensor("K", [S, D], F32, kind="ExternalInput").ap()
    Vt = nc.dram_tensor("V", [S, D], F32, kind="ExternalInput").ap()
    Ws = {w: nc.dram_tensor(w, [D, D], F32, kind="ExternalInput").ap()
          for w in ("Wq", "Wk", "Wv", "Wo")}
    bs = {b_: nc.dram_tensor(b_, [D], F32, kind="ExternalInput").ap()
          for b_ in ("bq", "bk", "bv", "bo")}
    attn_c = nc.dram_tensor("attn_c", [H, SQ, S], F32, kind="ExternalOutput").ap()
    out_c = nc.dram_tensor("out_c", [SQ, D], F32, kind="ExternalOutput").ap()

    with tile.TileContext(nc) as tc:
        with tc.tile_pool(name="persist", bufs=1) as persist:
            pKTb = persist.tile([128, DT, S], BF16)      # [d-in, hdh-out, k]
            pVb = persist.tile([128, KTN, PVW], BF16)    # [k-in, k-out, h*(65)]
            pQTb = persist.tile([128, DT, SQ], BF16)
            WoTb = persist.tile([128, DT, D], BF16)      # [hdh-in, hdh-out, dout]
            OTall = persist.tile([128, DT, SQ], BF16)    # [hdh-in, hdh-out, q]
            ones_bf = persist.tile([1, 512], BF16)
            brow = {b_: persist.tile([1, D], BF16, name=f"brow_{b_}")
                    for b_ in ("bq", "bk", "bv", "bo")}

            nc.vector.memset(ones_bf, 1.0)
            # ones columns interleaved in pV: pVb[:, :, h*65 + 64] = 1
            nc.vector.memset(
                pVb.rearrange("p k (h c) -> p k h c", c=DH + 1)[:, :, :, DH], 1.0)

            # ---- phase 0-2: projections ----
            with tc.tile_pool(name="wtmp", bufs=3) as wtmp, \
                 tc.tile_pool(name="wT", bufs=1) as wT, \
                 tc.tile_pool(name="kvc", bufs=2) as kvc, \
                 tc.tile_pool(name="pp", bufs=4, space="PSUM") as pp:

                # bias rows -> bf16 [1, D]
                for b_ in ("bq", "bk", "bv", "bo"):
                    bf = wtmp.tile([1, D], F32, tag="brow_f32")
                    nc.sync.dma_start(out=bf, in_=bs[b_].rearrange("(a d) -> a d", a=1))
                    nc.vector.tensor_copy(brow[b_], bf)

                # W transposes: WT[p, t, c] = W[c, 128 t + p]
                WTs = {}
                for w in ("Wq", "Wk", "Wv"):
                    WTs[w] = wT.tile([128, DT, D], BF16, tag=f"{w}T", name=f"{w}T")
                WTs["Wo"] = WoTb
                for w in ("Wq", "Wk", "Wv", "Wo"):
                    for r in range(DT):
                        wf = wtmp.tile([128, D], F32, tag="w_f32")
                        nc.sync.dma_start(out=wf, in_=Ws[w][r * 128:(r + 1) * 128, :])
                        wb = wtmp.tile([128, D], BF16, tag="w_bf16")
                        nc.vector.tensor_copy(wb, wf)
                        nc.sync.dma_start_transpose(
                            out=WTs[w][:, :, r * 128:(r + 1) * 128], in_=wb)

                # Q -> QTb [128, DT, SQ]
                QTb = wT.tile([128, DT, SQ], BF16, tag="QTb")
                for i in range(QT):
                    qf = wtmp.tile([128, D], F32, tag="w_f32")
                    nc.sync.dma_start(out=qf, in_=Qc[i * 128:(i + 1) * 128, :])
                    qb = wtmp.tile([128, D], BF16, tag="w_bf16")
                    nc.vector.tensor_copy(qb, qf)
                    nc.sync.dma_start_transpose(
                        out=QTb[:, :, i * 128:(i + 1) * 128], in_=qb)

                # pQT = Wq^T-contraction + bq
                for hb in range(DT):
                    ps = pp.tile([128, SQ], F32, tag="proj")
                    nc.tensor.matmul(ps, brow["bq"][:, hb * 128:(hb + 1) * 128],
                                     ones_bf[:, :SQ], start=True, stop=False)
                    for t in range(DT):
                        nc.tensor.matmul(ps, WTs["Wq"][:, t, hb * 128:(hb + 1) * 128],
                                         QTb[:, t, :], start=False, stop=(t == DT - 1))
                    nc.vector.tensor_copy(pQTb[:, hb, :], ps)

                # K/V streamed per 512-chunk
                for kc in range(KCN):
                    KTc = kvc.tile([128, DT, 512], BF16, tag="KTc")
                    VTc = kvc.tile([128, DT, 512], BF16, tag="VTc")
                    for i in range(4):
                        r0 = kc * 512 + i * 128
                        for (src, dst) in ((Kt, KTc), (Vt, VTc)):
                            xf = wtmp.tile([128, D], F32, tag="w_f32")
                            nc.sync.dma_start(out=xf, in_=src[r0:r0 + 128, :])
                            xb = wtmp.tile([128, D], BF16, tag="w_bf16")
                            nc.vector.tensor_copy(xb, xf)
                            nc.sync.dma_start_transpose(
                                out=dst[:, :, i * 128:(i + 1) * 128], in_=xb)
                    # pKT chunk
                    for hb in range(DT):
                        ps = pp.tile([128, 512], F32, tag="proj")
                        nc.tensor.matmul(ps, brow["bk"][:, hb * 128:(hb + 1) * 128],
                                         ones_bf[:, :512], start=True, stop=False)
                        for t in range(DT):
                            nc.tensor.matmul(ps, WTs["Wk"][:, t, hb * 128:(hb + 1) * 128],
                                             KTc[:, t, :], start=False, stop=(t == DT - 1))
                        nc.vector.tensor_copy(
                            pKTb[:, hb, kc * 512:(kc + 1) * 512], ps)
                    # pV chunk (4 k-subtiles x 2 hdh-halves)
                    pVb4 = pVb.rearrange("p k (h c) -> p k h c", c=DH + 1)
                    for ki in range(4):
                        for nb in range(2):
                            ps = pp.tile([128, 384], F32, tag="pvproj")
                            nc.tensor.matmul(ps, ones_bf[:, :128],
                                             brow["bv"][:, nb * 384:(nb + 1) * 384],
                                             start=True, stop=False)
                            for t in range(DT):
                                nc.tensor.matmul(
                                    ps, VTc[:, t, ki * 128:(ki + 1) * 128],
                                    WTs["Wv"][:, t, nb * 384:(nb + 1) * 384],
                                    start=False, stop=(t == DT - 1))
                            nc.vector.tensor_copy(
                                pVb4[:, kc * 4 + ki, nb * 6:(nb + 1) * 6, 0:DH], ps)

            # ---- phase 3: attention per head ----
            with tc.tile_pool(name="sc", bufs=2, space="PSUM") as scp, \
                 tc.tile_pool(name="ot", bufs=2, space="PSUM") as otp, \
                 tc.tile_pool(name="ut", bufs=3) as utp, \
                 tc.tile_pool(name="u", bufs=2) as up, \
                 tc.tile_pool(name="small", bufs=4) as small, \
                 tc.tile_pool(name="dsc", bufs=2, space="DRAM") as dscp:
                for h in range(H):
                    po = (h % 2) * 64
                    hb = h // 2
                    pq = pQTb[po:po + 64, hb, :]  # [64, SQ]

                    # --- A: UT tiles + attn@V with denominators ---
                    ot_ps = otp.tile([DH + 1, SQ], F32, tag="ot")
                    for (k0, gl) in _groups(KTN, SC_G):
                        sc_ps = scp.tile([128, SC_G * 512], F32, tag="sc")
                        ut_sb = utp.tile([128, SC_G * 512], BF16, tag="ut")
                        for j in range(gl):
                            kt = k0 + j
                            nc.tensor.matmul(
                                sc_ps[:, j * 512:(j + 1) * 512],
                                pKTb[po:po + 64, hb, kt * 128:(kt + 1) * 128],
                                pq, start=True, stop=True)
                        nc.scalar.activation(ut_sb[:, :gl * 512], sc_ps[:, :gl * 512],
                                             EXP, scale=float(1.0 / DH))
                        for j in range(gl):
                            kt = k0 + j
                            nc.tensor.matmul(
                                ot_ps, pVb[:, kt, h * 65:(h + 1) * 65],
                                ut_sb[:, j * 512:(j + 1) * 512],
                                start=(kt == 0), stop=(kt == KTN - 1))
                    rrow = small.tile([1, SQ], F32, tag="rrow")
                    nc.vector.reciprocal(rrow, ot_ps[DH:DH + 1, :])
                    rdram = dscp.tile([1, SQ], F32, tag="rdram")
                    nc.sync.dma_start(out=rdram, in_=rrow)
                    bcast = small.tile([64, SQ], F32, tag="bcast")
                    nc.sync.dma_start(out=bcast, in_=rdram.partition_broadcast(64))
                    nc.vector.tensor_mul(OTall[po:po + 64, hb, :],
                                         ot_ps[0:DH, :], bcast)

                    # --- B: attn rows + DMA out ---
                    for qt in range(QT):
                        u_sb = up.tile([128, S], F32, tag="u")
                        parts = small.tile([128, 4], F32, tag="parts")
                        gs = _groups(KCN, SC_G)
                        for gi, (c0, gl) in enumerate(gs):
                            sc_ps = scp.tile([128, SC_G * 512], F32, tag="sc")
                            for j in range(gl):
                                c = c0 + j
                                nc.tensor.matmul(
                                    sc_ps[:, j * 512:(j + 1) * 512],
                                    pQTb[po:po + 64, hb, qt * 128:(qt + 1) * 128],
                                    pKTb[po:po + 64, hb, c * 512:(c + 1) * 512],
                                    start=True, stop=True)
                            nc.scalar.activation(
                                u_sb[:, c0 * 512:(c0 + gl) * 512],
                                sc_ps[:, :gl * 512], EXP, scale=float(1.0 / DH),
                                accum_out=parts[:, gi:gi + 1])
                        sums = small.tile([128, 1], F32, tag="sums")
                        nc.vector.reduce_sum(sums, parts[:, 0:len(gs)],
                                             axis=mybir.AxisListType.X)
                        recip = small.tile([128, 1], F32, tag="recip")
                        nc.vector.reciprocal(recip, sums)
                        nc.vector.tensor_scalar_mul(u_sb, u_sb, recip)
                        nc.sync.dma_start(
                            out=attn_c[h, qt * 128:(qt + 1) * 128, :], in_=u_sb)

            # ---- phase 4: output projection ----
            with tc.tile_pool(name="p4", bufs=4, space="PSUM") as p4, \
                 tc.tile_pool(name="o4", bufs=2) as o4:
                for qt in range(QT):
                    osb = o4.tile([128, D], F32, tag="osb")
                    for nb in range(2):
                        ps = p4.tile([128, 384], F32, tag="p4")
                        nc.tensor.matmul(ps, ones_bf[:, :128],
                                         brow["bo"][:, nb * 384:(nb + 1) * 384],
                                         start=True, stop=False)
                        for t in range(DT):
                            nc.tensor.matmul(
                                ps, OTall[:, t, qt * 128:(qt + 1) * 128],
                                WoTb[:, t, nb * 384:(nb + 1) * 384],
                                start=False, stop=(t == DT - 1))
                        nc.vector.tensor_copy(osb[:, nb * 384:(nb + 1) * 384], ps)
                    nc.sync.dma_start(out=out_c[qt * 128:(qt + 1) * 128, :], in_=osb)

    nc.compile()
    return nc


class _Runner:
    """Caches the compiled SPMD executable; mirrors bass2jax.run_bass_via_pjrt
    multi-core path but keeps the jitted function/device buffers reusable so
    repeat calls only pay input upload + execution + output fetch."""

    def __init__(self):
        import jax
        import jax.numpy as jnp
        from jax.sharding import Mesh, PartitionSpec
        from jax.experimental.shard_map import shard_map
        from concourse import bass2jax
        from concourse import mybir as _mybir

        self.jax = jax
        nc = build_nc()
        bass2jax.install_neuronx_cc_hook()

        partition_name = (nc.partition_id_tensor.name
                          if nc.partition_id_tensor else None)
        in_names, out_names, out_avals, zero_shapes = [], [], [], []
        for alloc in nc.m.functions[0].allocations:
            if not isinstance(alloc, _mybir.MemoryLocationSet):
                continue
            name = alloc.memorylocations[0].name
            if alloc.kind == "ExternalInput":
                if name != partition_name:
                    in_names.append(name)
            elif alloc.kind == "ExternalOutput":
                out_names.append(name)
                shape = tuple(alloc.tensor_shape)
                dtype = _mybir.dt.np(alloc.dtype)
                out_avals.append(jax.core.ShapedArray(shape, dtype))
                zero_shapes.append((shape, dtype))
        self.in_names = list(in_names)
        self.out_names = list(out_names)
        self.out_avals = out_avals
        n_params = len(in_names)
        n_outs = len(out_names)
        all_in_names = in_names + out_names
        if partition_name is not None:
            all_in_names = all_in_names + [partition_name]

        def _body(*args):
            operands = list(args)
            if partition_name is not None:
                operands.append(bass2jax.partition_id_tensor())
            outs = bass2jax._bass_exec_p.bind(
                *operands,
                out_avals=tuple(out_avals),
                in_names=tuple(all_in_names),
                out_names=tuple(out_names),
                lowering_input_output_aliases=(),
                sim_require_finite=True,
                sim_require_nnan=True,
                nc=nc,
            )
            return tuple(outs)

        devices = jax.devices()[:CORES]
        self.mesh = Mesh(np.asarray(devices), ("core",))
        spec = PartitionSpec("core")
        self.sharding = jax.sharding.NamedSharding(self.mesh, spec)
        in_specs = (spec,) * (n_params + n_outs)
        out_specs = (spec,) * n_outs
        donate = tuple(range(n_params, n_params + n_outs))
        self.fn = jax.jit(
            shard_map(_body, mesh=self.mesh, in_specs=in_specs,
                      out_specs=out_specs, check_rep=False),
            donate_argnums=donate, keep_unused=True)

        def _mk_zeros():
            return tuple(
                jnp.zeros((CORES * sh[0], *sh[1:]), dt)
                for sh, dt in zero_shapes)

        self.mk_zeros = jax.jit(_mk_zeros,
                                out_shardings=(self.sharding,) * n_outs)

    def put_inputs(self, in_maps):
        """in_maps: list of CORES dicts name->np array. Returns device arrays."""
        jax = self.jax
        concat = [
            np.concatenate([np.asarray(in_maps[c][n]) for c in range(CORES)], axis=0)
            for n in self.in_names
        ]
        return [jax.device_put(a, self.sharding) for a in concat]

    def run(self, in_dev):
        zeros = self.mk_zeros()
        outs = self.fn(*in_dev, *zeros)
        return outs


_RUNNER = None


def _get_runner():
    global _RUNNER
    if _RUNNER is None:
        _RUNNER = _Runner()
    return _RUNNER


def _make_in_maps(Q, K, V, Wq, bq, Wk, bk, Wv, bv, Wo, bo):
    Q = np.ascontiguousarray(np.asarray(Q, dtype=np.float32)).reshape(S, D)
    shared = {
        "K": np.ascontiguousarray(np.asarray(K, np.float32).reshape(S, D)),
        "V": np.ascontiguousarray(np.asarray(V, np.float32).reshape(S, D)),
        "Wq": np.ascontiguousarray(np.asarray(Wq, np.float32)),
        "Wk": np.ascontiguousarray(np.asarray(Wk, np.float32)),
        "Wv": np.ascontiguousarray(np.asarray(Wv, np.float32)),
        "Wo": np.ascontiguousarray(np.asarray(Wo, np.float32)),
        "bq": np.ascontiguousarray(np.asarray(bq, np.float32)),
        "bk": np.ascontiguousarray(np.asarray(bk, np.float32)),
        "bv": np.ascontiguousarray(np.asarray(bv, np.float32)),
        "bo": np.ascontiguousarray(np.asarray(bo, np.float32)),
    }
    in_maps = []
    for c in range(CORES):
        m = dict(shared)
        m["Qc"] = np.ascontiguousarray(Q[c * SQ:(c + 1) * SQ, :])
        in_maps.append(m)
    return in_maps


def kernel(Q, K, V, Wq, bq, Wk, bk, Wv, bv, Wo, bo):
    r = _get_runner()
    in_dev = r.put_inputs(_make_in_maps(Q, K, V, Wq, bq, Wk, bk, Wv, bv, Wo, bo))
    outs = r.run(in_dev)
    by_name = dict(zip(r.out_names, outs))
    attn_g = np.asarray(by_name["attn_c"])  # [CORES*H, SQ, S]
    out_g = np.asarray(by_name["out_c"])    # [CORES*SQ, D]
    out = out_g.reshape(1, S, D)
    attn = np.ascontiguousarray(
        attn_g.reshape(CORES, H, SQ, S).transpose(1, 0, 2, 3).reshape(1, H, S, S))
    return (out, attn)


if __name__ == "__main__":
    nc = build_nc()
    print("built+compiled ok")
